# revision 15
# baseline (speedup 1.0000x reference)
"""Exact KNN collision kernel for trn2 (8 NeuronCores).

Computes nn[b,n] = argmin_m |vertices[b,n] - collider[b, cvi[m]]|^2 matching the
reference's fp32 arithmetic, with first-occurrence tie-breaking.

Per core (core c -> batch b=c//2, row-half h=c%2, 8192 rows, 64 tiles of 128):
  - host dedups gathered collider points (first-occurrence order preserved)
  - PE: K=31 bf16 matmuls per tile compute, in fp32 PSUM,
        s = sum of exact bf16-split products of v.c plus (-|c|^2/2)
    (v and c split into 3 bf16 terms each -> 9 pair groups x 3 dims; the
     -c2/2 term enters as 4 bf16 rows against a ones stationary row).
  - the row is processed as 2 independent column pieces; per piece i:
      ACT copies the PSUM piece to SBUF (fast PSUM recycling)
      DVE (2x SBUF mode): m_i = max(s_i), then a single weighted reduction
          acc_i = sum_{s_c > m_i - theta} (U + c)   (w[c] = U + c, exact fp32)
      acc encodes both the near-max candidate count g = acc div U and, when
      g == 1, the argmax column j = acc - U.
  - host picks the winning piece (max m_i, first on ties); rows with g >= 2
    or two live pieces may disagree with the fp32 reference and are
    re-resolved on host via the reference formula (jax einsum on device).
"""
import os
import sys
import numpy as np

_BASS_PATH = "/opt/trn_rl_repo"
if _BASS_PATH not in sys.path:
    sys.path.insert(0, _BASS_PATH)

import ml_dtypes

B, N, V, M = 4, 16384, 6890, 4096
NCORES = 8
ROWS = (B * N) // NCORES          # 8192 rows per core
NT = ROWS // 128                  # 64 row tiles
KROWS = 31                        # 27 product rows + 4 t rows
THETA = 8e-6                      # ambiguity threshold (validated on data)

_BF = ml_dtypes.bfloat16
_PROGRAM_CACHE = {}


def _pieces_of(U):
    half = (U + 1) // 2
    return [(0, half), (half, U)]


def _build_program(U):
    import concourse.bacc as bacc
    import concourse.mybir as mybir
    import concourse.tile as tile

    f32 = mybir.dt.float32
    bf16 = mybir.dt.bfloat16

    pieces = _pieces_of(U)
    NP = len(pieces)
    P0 = pieces[0][1]

    nc = bacc.Bacc("TRN2", target_bir_lowering=False, debug=False,
                   num_devices=NCORES)
    stat = nc.dram_tensor("stat", [KROWS, ROWS], bf16, kind="ExternalInput")
    mov = nc.dram_tensor("mov", [KROWS, U], bf16, kind="ExternalInput")
    wrow_d = nc.dram_tensor("wrow", [128, U], f32, kind="ExternalInput")
    o_ws = nc.dram_tensor("ws", [128, NT], f32, kind="ExternalOutput")

    with tile.TileContext(nc) as tc:
        with (
            tc.tile_pool(name="const", bufs=1) as cpool,
            tc.tile_pool(name="work", bufs=3) as wpool,
            tc.tile_pool(name="junkp", bufs=1) as jpool,
            tc.tile_pool(name="psum", bufs=2, space="PSUM") as ppool,
        ):
            stat_sb = cpool.tile([KROWS, ROWS], bf16)
            mov_sb = cpool.tile([KROWS, U], bf16)
            wrow = cpool.tile([128, U], f32)
            # first tile's operands first: mov + stat tile 0, then the rest
            nc.sync.dma_start(mov_sb[:], mov[:])
            nc.sync.dma_start(stat_sb[:, 0:128], stat[:, 0:128])
            nc.sync.dma_start(stat_sb[:, 128:ROWS], stat[:, 128:ROWS])
            nc.sync.dma_start(wrow[:], wrow_d[:])

            ws = cpool.tile([128, NT], f32)
            junk1 = jpool.tile([128, U], f32)
            junk2 = jpool.tile([128, U], f32)

            for t in range(NT):
                st = stat_sb[:, t * 128:(t + 1) * 128]
                s_sb = wpool.tile([128, U], f32, tag="s")
                madj = wpool.tile([128, 1], f32, tag="madj")
                for pi, (lo, hi) in enumerate(pieces):
                    w = hi - lo
                    ps = ppool.tile([128, P0], f32, tag="ps")
                    q = lo
                    while q < hi:
                        qe = min(q + 512, hi)
                        nc.tensor.matmul(ps[:, q - lo:qe - lo], st,
                                         mov_sb[:, q:qe], start=True, stop=True)
                        q = qe
                    # drain PSUM fast: ACT copy piece -> SBUF
                    nc.scalar.copy(s_sb[:, lo:hi], ps[:, 0:w])
                # fused madj pass: accum = max(s - theta)  (DVE 2x from SBUF)
                if t == 0:
                    # tile-0 fast path: per-piece partial maxes start right
                    # after the first piece's copy, then a tiny combine
                    madj2 = wpool.tile([128, 2], f32, tag="madj2")
                    for pi, (lo, hi) in enumerate(pieces):
                        nc.vector.tensor_scalar(
                            junk1[:, lo:hi], s_sb[:, lo:hi], -THETA, None,
                            op0=mybir.AluOpType.add, op1=mybir.AluOpType.max,
                            accum_out=madj2[:, pi:pi + 1])
                    nc.vector.tensor_scalar(
                        junk1[:, 0:2], madj2[:], 0.0, None,
                        op0=mybir.AluOpType.add, op1=mybir.AluOpType.max,
                        accum_out=madj[:])
                else:
                    nc.vector.tensor_scalar(
                        junk1[:], s_sb[:], -THETA, None,
                        op0=mybir.AluOpType.add, op1=mybir.AluOpType.max,
                        accum_out=madj[:])
                # weighted near-max sum: acc = sum_{s_c > m - theta} (U + c)
                #   g = acc // U  (candidate count), j = acc - U when g == 1
                nc.vector.scalar_tensor_tensor(
                    junk2[:], s_sb[:], madj[:], wrow[:],
                    op0=mybir.AluOpType.is_gt, op1=mybir.AluOpType.mult,
                    accum_out=ws[:, t:t + 1])
                if t == NT - 9:
                    nc.sync.dma_start(o_ws[:, 0:NT - 8], ws[:, 0:NT - 8])

            nc.sync.dma_start(o_ws[:, NT - 8:NT], ws[:, NT - 8:NT])
    nc.compile()
    return nc


def _get_program(U):
    if U not in _PROGRAM_CACHE:
        _PROGRAM_CACHE[U] = _build_program(U)
    return _PROGRAM_CACHE[U]


def _split3(x):
    """Exact 3-term bf16 split of fp32 (24 mantissa bits = 3 x 8)."""
    h = x.astype(_BF).astype(np.float32)
    r = x - h
    m = r.astype(_BF).astype(np.float32)
    r2 = r - m
    l = r2.astype(_BF).astype(np.float32)
    return h, m, l


def kernel(vertices, collider, collision_vertices, _want_trace=False):
    from concourse.bass_utils import run_bass_kernel_spmd

    v = np.ascontiguousarray(np.asarray(vertices), dtype=np.float32)   # [B,N,3]
    c = np.ascontiguousarray(np.asarray(collider), dtype=np.float32)   # [B,V,3]
    cvi = np.asarray(collision_vertices).astype(np.int64)              # [M]

    # dedup candidates, keeping first-occurrence order (exact tie semantics)
    u, first_pos = np.unique(cvi, return_index=True)
    order = np.argsort(first_pos)
    u = u[order]
    first_pos = first_pos[order].astype(np.int32)
    U = len(u)
    pieces = _pieces_of(U)
    NP = len(pieces)

    wrow_full = np.ascontiguousarray(
        np.broadcast_to((np.arange(U, dtype=np.float64) + U)
                        .astype(np.float32)[None, :], (128, U)))
    in_maps = []
    mov_cache = {}
    for core in range(NCORES):
        b = core // 2
        r0 = (core % 2) * ROWS
        if b not in mov_cache:
            cb = c[b][u]                                               # [U,3]
            c2 = ((cb[:, 0] * cb[:, 0] + cb[:, 1] * cb[:, 1])
                  + cb[:, 2] * cb[:, 2]).astype(np.float32)
            t32 = (-(c2) * np.float32(0.5)).astype(np.float32)
            th = t32.astype(_BF).astype(np.float32)
            r = t32 - th
            tm = r.astype(_BF).astype(np.float32)
            r2 = r - tm
            tl = r2.astype(_BF).astype(np.float32)
            t3 = (r2 - tl).astype(_BF).astype(np.float32)
            ch, cm, cl = _split3(cb)
            movrows = []
            statsel = []
            for (sv, sc_) in [("h", ch), ("h", cm), ("m", ch), ("h", cl),
                              ("m", cm), ("l", ch), ("m", cl), ("l", cm),
                              ("l", cl)]:
                for d in range(3):
                    movrows.append(sc_[:, d])
                    statsel.append((sv, d))
            for tt in (th, tm, tl, t3):
                movrows.append(tt)
                statsel.append(("1", 0))
            mov = np.stack(movrows, axis=0).astype(_BF)                 # [31,U]
            mov_cache[b] = (mov, statsel)
        mov, statsel = mov_cache[b]
        vb = v[b, r0:r0 + ROWS, :]                                      # [ROWS,3]
        vh, vm, vl = _split3(vb)
        vparts = {"h": vh, "m": vm, "l": vl}
        statrows = []
        for (sv, d) in statsel:
            if sv == "1":
                statrows.append(np.ones(ROWS, np.float32))
            else:
                statrows.append(vparts[sv][:, d])
        statm = np.stack(statrows, axis=0).astype(_BF)                  # [31,ROWS]
        in_maps.append({"stat": statm, "mov": np.ascontiguousarray(mov),
                        "wrow": wrow_full})

    nc = _get_program(U)
    res = run_bass_kernel_spmd(nc, in_maps, core_ids=list(range(NCORES)))

    nn = np.zeros((B, N), np.int32)
    flags = np.zeros((B, N), bool)
    for core in range(NCORES):
        b = core // 2
        r0 = (core % 2) * ROWS
        rr = res.results[core]
        # [128, NT] -> [ROWS(NT*128)]
        acc = np.rint(rr["ws"].T.reshape(-1)).astype(np.int64)
        # acc = g*U + sum of matching column ids; g==1 <=> acc < 2U
        j = np.clip(acc - U, 0, U - 1)
        nn[b, r0:r0 + ROWS] = first_pos[j]
        flags[b, r0:r0 + ROWS] = acc >= 2 * U

    # host repair of ambiguous rows: run the reference formula (same device
    # fp32 path) for just the flagged rows of each batch
    if flags.any():
        import jax.numpy as jnp
        for b in range(B):
            rows = np.nonzero(flags[b])[0]
            if len(rows) == 0:
                continue
            cv = jnp.take(jnp.asarray(c[b]), jnp.asarray(cvi.astype(np.int32)),
                          axis=-2)                                      # [M,3]
            d2 = (jnp.sum(cv * cv, axis=-1)[None, :]
                  - 2.0 * jnp.einsum('nd,md->nm',
                                     jnp.asarray(v[b, rows]), cv))
            nn[b, rows] = np.asarray(jnp.argmin(d2, axis=-1).astype(jnp.int32))

    batch_idx = np.broadcast_to(np.arange(B, dtype=np.int32)[:, None], nn.shape)
    outv = np.stack([batch_idx, nn], axis=-1).astype(np.int32)
    if _want_trace:
        return outv, (res, in_maps)
    return outv
